# revision 56
# baseline (speedup 1.0000x reference)
"""3-layer GCN (DiffPool-style conv stack) on Trainium2, 8 NeuronCores.

Strategy (graph/data parallel, per sharding hint):
  - Nodes are permuted by degree and dealt round-robin to 8 cores
    (12544 local nodes each incl. dummy padding; 98 blocks of 128).
  - Edges partitioned by destination owner; per core the edge stream is
    grouped by src quadrant and densely packed in dst-block order (a
    gather tile may span blocks; each (tile, block) intersection is a
    "part" with its own one-hot column), so dma_gather indices fit int16.
  - Per layer: each core computes the table rows for its own nodes
    T = dinv * (H @ W) (node-major, bf16), FOUR quarter-AllGathers
    assemble the full table (each fires as soon as its quarter of the
    tails is written, hiding the collective under the gather stream),
    then per-edge rows are fetched with gpsimd dma_gather (1024
    idxs/call, single_packet, 4 SWDGE queues, 32KB/partition descriptor
    rings) and aggregated with one-hot selection matmuls into per-block
    PSUM, accumulated across quadrants in an SBUF slab, then
    scale+relu'd (fused, Scalar engine) into the next layer's input;
    bias enters PSUM via a rank-1 matmul.
"""

import sys
import types

sys.path.insert(0, "/opt/trn_rl_repo")

import numpy as np

N = 100000
C = 128
NC = 8
L = 12544           # local nodes per core (98 blocks of 128)
B = L // 128        # 98
NPAD = NC * L       # 100352
QUADS = 4
QROWS = NPAD // QUADS   # 25088 (< 32767, fits int16 gather index)
CALL_MAX_TILES = 8      # 1024 indices per call: single_packet needs <=64 descs/engine, and multi-packet drains slower
N_QUEUES = 4

import ml_dtypes

TBL_NP = ml_dtypes.bfloat16  # table dtype; np.float32 or ml_dtypes.bfloat16


def _install_axon_profile_hook():
    """run_bass_kernel_spmd(trace=True) needs antenv.axon_hooks, absent in
    this image; register the equivalent ctypes hook."""
    try:
        import antenv
        if getattr(antenv, "axon_hooks", None) is not None:
            return
        from trn_agent_boot.trn_boot import _ntff_profile_via_ctypes
        mod = types.ModuleType("antenv.axon_hooks")
        hook = _ntff_profile_via_ctypes("/opt/axon/libaxon_pjrt.so")
        mod.get_axon_ntff_profile_hook = lambda: hook
        mod.set_axon_ntff_profile_hook = lambda h: None
        sys.modules["antenv.axon_hooks"] = mod
        antenv.axon_hooks = mod
    except Exception:
        pass


# ----------------------------------------------------------------------------
# Host preprocessing
# ----------------------------------------------------------------------------

def preprocess(x, edge_index):
    """Build the static SPMD schedule + per-core input arrays."""
    x = np.asarray(x, np.float32)
    ei = np.asarray(edge_index, np.int64)
    # self-loops are NOT placed in the gather stream: each core owns its
    # nodes' table rows, so the self term dinv_i^2*(HW)_i is added on-chip
    # from a stashed copy (identity matmul). deg still counts them.
    src = ei[0]
    dst = ei[1]

    deg = (np.bincount(dst, minlength=N) + 1).astype(np.float32)
    dinv = (1.0 / np.sqrt(deg)).astype(np.float32)

    order = np.argsort(deg, kind="stable")
    rank = np.empty(N, np.int64)
    rank[order] = np.arange(N)
    core_of = rank % NC
    # stratified snake-deal of each core's degree-ordered nodes over its
    # blocks: every block samples the same degree profile, equalizing
    # per-(quad, block) edge counts across blocks AND cores (minimizes
    # gather-tile padding, which is pure wasted gather bandwidth).
    pos = rank // NC
    slot_of = (pos % B) * 128 + pos // B
    gnew = core_of * L + slot_of

    # original node for (core, slot); -1 for dummy slots
    node_at = -np.ones((NC, L), np.int64)
    node_at[core_of, slot_of] = np.arange(N)

    gsrc = gnew[src]
    gdst = gnew[dst]
    owner = gdst // L
    ldst = gdst % L
    # table row numbering: quarter-shard interleave so the table is
    # assembled by FOUR AllGathers (one per quad) that fire as soon as each
    # quarter of the tails is done and fully pipeline with the gather phase.
    #   quarter j = slot // QTR; row = j*QROWS + core*QTR + slot%QTR
    QTR = L // 4                       # 3136 rows per quarter-shard
    sc = gsrc // L
    ss = gsrc % L
    qj = ss // QTR
    trow = qj * QROWS + sc * QTR + (ss - qj * QTR)
    quad = trow // QROWS
    qidx = trow % QROWS
    blk = ldst // 128
    sid = ldst % 128

    # dense per-quad packing: per-(quad, block) segments padded only to the
    # max-over-cores count (no 128 rounding); tiles may span blocks. Each
    # (tile, block) intersection is a "part" carrying its own one-hot column.
    key = (owner * QUADS + quad) * B + blk
    cnt = np.bincount(key, minlength=NC * QUADS * B).reshape(NC, QUADS, B)
    seg_len = cnt.max(axis=0).astype(np.int64)          # [QUADS, B]

    seg_start = np.zeros((QUADS, B), np.int64)          # slot offset in quad
    quad_tiles = np.zeros(QUADS, np.int64)
    quad_tile0 = np.zeros(QUADS, np.int64)
    t = 0
    for q in range(QUADS):
        s = 0
        for b in range(B):
            seg_start[q, b] = s
            s += seg_len[q, b]
        quad_tile0[q] = t
        quad_tiles[q] = (s + 127) // 128
        t += quad_tiles[q]
    n_tiles = int(t)
    S = n_tiles * 128

    # parts ordered by (q, tile, b)
    part_first = np.zeros((QUADS, B), np.int64)
    t_lo_g = np.zeros((QUADS, B), np.int64)
    parts_by_tile = [[] for _ in range(n_tiles)]  # (part, b, first, last)
    np_idx = 0
    for q in range(QUADS):
        for b in range(B):
            st = int(seg_start[q, b])
            en = st + int(seg_len[q, b])
            en_t = int(quad_tiles[q]) * 128 if b == B - 1 else en
            lo = st // 128
            hi = (en_t - 1) // 128
            part_first[q, b] = np_idx
            t_lo_g[q, b] = quad_tile0[q] + lo
            for tt in range(lo, hi + 1):
                gt = int(quad_tile0[q]) + tt
                parts_by_tile[gt].append((np_idx, b, tt == lo, tt == hi))
                np_idx += 1
    n_parts = np_idx

    # calls: chunk each quad's tile range into <=CALL_MAX_TILES-tile calls;
    # per call, the covered parts [(part_local, tile_local, b, first, last)]
    calls = []
    for q in range(QUADS):
        off = 0
        while off < int(quad_tiles[q]):
            ntl = min(CALL_MAX_TILES, int(quad_tiles[q]) - off)
            gt0 = int(quad_tile0[q]) + off
            plist = []
            p0 = parts_by_tile[gt0][0][0]
            for tt in range(ntl):
                for (pi, b, first, last) in parts_by_tile[gt0 + tt]:
                    plist.append((pi - p0, tt, b, first, last))
            calls.append((q, gt0, ntl, p0, plist))
            off += ntl
    n_calls = len(calls)
    parts_call_max = max(len(c[4]) for c in calls)

    # per-core slot arrays; pad slots gather a valid (spread) row but carry
    # sid=-999 so their one-hot column is all zeros.
    eorder = np.lexsort((qidx, blk, quad, owner))
    so, sq, sb_, sqi, ssid = (owner[eorder], quad[eorder], blk[eorder],
                              qidx[eorder], sid[eorder])
    skey = key[eorder]
    grp_change = np.flatnonzero(np.diff(skey, prepend=-1))
    grp_starts = np.zeros(len(skey), np.int64)
    grp_starts[grp_change] = np.arange(len(skey))[grp_change]
    np.maximum.accumulate(grp_starts, out=grp_starts)
    ranks = np.arange(len(skey)) - grp_starts

    slot = quad_tile0[sq] * 128 + seg_start[sq, sb_] + ranks
    gt_e = slot // 128
    part_e = part_first[sq, sb_] + (gt_e - t_lo_g[sq, sb_])

    pad_rows = (np.arange(S, dtype=np.int64) * 97) % QROWS
    idx16 = np.tile(pad_rows.astype(np.int16)[None, :], (NC, 1))
    idx16[so, slot] = sqi.astype(np.int16)

    sidc = np.full((NC, 128, n_parts), -999.0, np.float32)
    sidc[so, slot % 128, part_e] = ssid.astype(np.float32)

    callcnt = np.tile(np.array([ntl * 128 for (_, _, ntl, _, _) in calls],
                               np.int32)[None, :], (NC, 1))

    # wrapped per-core arrays
    idx_wr = np.zeros((NC, 128, S // 16), np.int16)
    for k in range(NC):
        w16 = idx16[k].reshape(S // 16, 16).T            # [16, S/16]
        idx_wr[k] = np.tile(w16, (8, 1))
    sid_wr = sidc                                        # [NC, 128, n_parts]

    # per-core node-major inputs
    xT = np.zeros((NC, 128, L), np.float32)
    dinv_wr = np.zeros((NC, 128, B), np.float32)
    for k in range(NC):
        nodes = node_at[k]
        real = nodes >= 0
        xk = np.zeros((L, C), np.float32)
        xk[real] = x[nodes[real]]
        xT[k] = xk.T
        dk = np.zeros(L, np.float32)
        dk[real] = dinv[nodes[real]]
        dinv_wr[k] = dk.reshape(B, 128).T

    return dict(
        node_at=node_at, dinv=dinv, S=S, n_tiles=n_tiles, n_parts=n_parts,
        calls=calls, n_calls=n_calls, parts_call_max=parts_call_max,
        idx16=idx16, sidc=sidc, callcnt=callcnt,
        idx_wr=idx_wr, sid_wr=sid_wr, xT=xT, dinv_wr=dinv_wr,
    )


def numpy_model(prep, x, Ws, bs, tbl_dt=None):
    """Exact numpy emulation of the device algorithm (for validation)."""
    if tbl_dt is None:
        tbl_dt = TBL_NP
    node_at = prep["node_at"]
    dinv_wr = prep["dinv_wr"]

    # dinv per (core, local) in node-major
    dloc = np.stack([dinv_wr[k].T.reshape(L) for k in range(NC)])   # [NC, L]
    H = np.stack([prep["xT"][k].T for k in range(NC)])              # [NC, L, C]

    out = None
    QTR = L // 4
    for l in range(3):
        # table build (quarter-shard interleave)
        table = np.zeros((NPAD, C), tbl_dt)
        own = []
        for k in range(NC):
            tk = ((H[k].astype(np.float32) @ Ws[l])
                  * dloc[k][:, None]).astype(tbl_dt)
            own.append(tk)
            for j in range(4):
                table[j * QROWS + k * QTR:j * QROWS + (k + 1) * QTR] = \
                    tk[j * QTR:(j + 1) * QTR]

        # aggregation (parts-based)
        Hn = np.zeros((NC, L, C), np.float32)
        for k in range(NC):
            S_acc = np.zeros((L, C), np.float32)
            for (q, gt0, ntl, p0, plist) in prep["calls"]:
                for (pl, tt, b, first, last) in plist:
                    gt = gt0 + tt
                    rows = (prep["idx16"][k][gt * 128:(gt + 1) * 128]
                            .astype(np.int64) + q * QROWS)
                    sidv = prep["sidc"][k][:, p0 + pl]
                    valid = sidv >= 0
                    tgt = b * 128 + sidv[valid].astype(np.int64)
                    np.add.at(S_acc, tgt,
                              table[rows[valid]].astype(np.float32))
            S_acc += own[k].astype(np.float32)          # self-loop term
            z = S_acc * dloc[k][:, None] + bs[l][None, :]
            Hn[k] = np.maximum(z, 0.0)
        H = Hn
        out = H
    # assemble
    full = np.zeros((N, C), np.float32)
    for k in range(NC):
        real = node_at[k] >= 0
        full[node_at[k][real]] = out[k][real]
    return full


# ----------------------------------------------------------------------------
# Bass program
# ----------------------------------------------------------------------------

def build_nc(prep, tbl_dt_np=None, debug_stage=None):
    import concourse.bass as bass
    import concourse.mybir as mybir
    import concourse.tile as tile
    from concourse import bacc

    if tbl_dt_np is None:
        tbl_dt_np = TBL_NP
    TBL_DT = mybir.dt.from_np(np.dtype(tbl_dt_np))
    F32 = mybir.dt.float32

    S = prep["S"]
    n_tiles = prep["n_tiles"]
    n_parts = prep["n_parts"]
    calls = prep["calls"]
    n_calls = prep["n_calls"]
    parts_call_max = prep["parts_call_max"]

    nc = bacc.Bacc("TRN2", target_bir_lowering=False, debug=False,
                   num_devices=NC, num_swdge_queues=N_QUEUES,
                   dynamic_dma_scratch_size=2**15)

    # inputs (xT and W in table dtype: the whole GEMM path runs bf16)
    xT_in = nc.dram_tensor("xT", [128, L], TBL_DT, kind="ExternalInput")
    w_in = [nc.dram_tensor(f"W{i+1}", [128, 128], TBL_DT,
                           kind="ExternalInput")
            for i in range(3)]
    bias_in = [nc.dram_tensor(f"Bt{i+1}", [128, 128], F32, kind="ExternalInput")
               for i in range(3)]
    iota_in = nc.dram_tensor("iota", [128, 128], TBL_DT, kind="ExternalInput")
    ident_in = nc.dram_tensor("ident", [128, 128], F32, kind="ExternalInput")
    identb_in = nc.dram_tensor("identb", [128, 128], TBL_DT,
                               kind="ExternalInput")
    e0b_in = nc.dram_tensor("e0b", [128, 128], TBL_DT, kind="ExternalInput")
    dinv_in = nc.dram_tensor("dinv", [128, B], F32, kind="ExternalInput")
    sid_in = nc.dram_tensor("sid", [128, n_parts], TBL_DT,
                            kind="ExternalInput")
    idx_in = nc.dram_tensor("idx", [128, S // 16], mybir.dt.int16,
                            kind="ExternalInput")
    out_dram = nc.dram_tensor("out", [L, 128], F32, kind="ExternalOutput")
    tbl_dbg_in = None
    slab_dbg = None
    if debug_stage == "agg_only":
        tbl_dbg_in = nc.dram_tensor("tbl_dbg", [NPAD, 128], TBL_DT,
                                    kind="ExternalInput")
        slab_dbg = nc.dram_tensor("slab_dbg", [L, 128], F32,
                                  kind="ExternalOutput")
        g_dbg = nc.dram_tensor("g_dbg", [128, CALL_MAX_TILES * 128], TBL_DT,
                               kind="ExternalOutput")
        a_dbg = nc.dram_tensor("a_dbg", [128, 128], F32,
                               kind="ExternalOutput")

    from contextlib import ExitStack

    with tile.TileContext(nc) as tc, ExitStack() as es:
        constp = es.enter_context(tc.tile_pool(name="const", bufs=1))
        idxp = es.enter_context(tc.tile_pool(name="idxp", bufs=1))
        xtp = es.enter_context(tc.tile_pool(name="xt", bufs=2))
        gatp = es.enter_context(tc.tile_pool(name="gat", bufs=14))
        app = es.enter_context(tc.tile_pool(name="ap", bufs=8))
        slabp = es.enter_context(tc.tile_pool(name="slab", bufs=B))
        workp = es.enter_context(tc.tile_pool(name="work", bufs=4))
        tblp = es.enter_context(tc.tile_pool(name="tblp", bufs=B + 8))
        htp = es.enter_context(tc.tile_pool(name="htp", bufs=3))
        aggps = es.enter_context(tc.tile_pool(name="aggps", bufs=5, space="PSUM"))
        tpps = es.enter_context(tc.tile_pool(name="tpps", bufs=1, space="PSUM"))
        gemmps = es.enter_context(tc.tile_pool(name="gemmps", bufs=2, space="PSUM"))
        dramp = es.enter_context(tc.tile_pool(name="dram", bufs=1, space="DRAM"))
        if True:

            # ---- resident constants ----
            w_sb = []
            bias_sb = []
            for i in range(3):
                w = constp.tile([128, 128], TBL_DT, tag=f"w{i}")
                nc.sync.dma_start(w[:], w_in[i][:, :])
                w_sb.append(w)
                bb = constp.tile([128, 128], F32, tag=f"bias{i}")
                nc.sync.dma_start(bb[:], bias_in[i][:, :])
                bias_sb.append(bb)
            iota_sb = constp.tile([128, 128], TBL_DT, tag="iota")
            nc.sync.dma_start(iota_sb[:], iota_in[:, :])
            ident_sb = constp.tile([128, 128], F32, tag="ident")
            nc.sync.dma_start(ident_sb[:], ident_in[:, :])
            identb_sb = constp.tile([128, 128], TBL_DT, tag="identb")
            nc.sync.dma_start(identb_sb[:], identb_in[:, :])
            # row-0-ones bf16 tile (rank-1 bias matmul: psum += e0^T @ biasb)
            e0b_sb = constp.tile([128, 128], TBL_DT, tag="e0b")
            nc.sync.dma_start(e0b_sb[:], e0b_in[:, :])
            biasb_sb = []
            for i in range(3):
                bbq = constp.tile([128, 128], TBL_DT, tag=f"biasb{i}")
                nc.vector.tensor_copy(bbq[:], bias_sb[i][:])
                biasb_sb.append(bbq)
            dinv_sb = constp.tile([128, B], F32, tag="dinv")
            nc.sync.dma_start(dinv_sb[:], dinv_in[:, :])
            sid_sb = constp.tile([128, n_parts], TBL_DT, tag="sid")
            nc.sync.dma_start(sid_sb[:], sid_in[:, :])
            idx_sb = idxp.tile([128, S // 16], mybir.dt.int16, tag="idx")
            nc.sync.dma_start(idx_sb[:], idx_in[:, :])

            QTR = L // 4               # 3136 rows per quarter-shard
            # last block whose slots fall (partly) in quarter j: AG-j fires
            # once that block's myshard rows are written.
            qtr_last_blk = [((j + 1) * QTR - 1) // 128 for j in range(4)]
            myshard_q = [dramp.tile([QTR, 128], TBL_DT, tag=f"myshard{j}",
                                    name=f"myshard{j}")
                         for j in range(4)]
            table_q = [dramp.tile([QROWS, 128], TBL_DT, tag=f"table{j}",
                                  name=f"table{j}")
                       for j in range(4)]

            def do_allgather(j):
                nc.gpsimd.collective_compute(
                    "AllGather",
                    mybir.AluOpType.bypass,
                    replica_groups=[list(range(NC))],
                    ins=[myshard_q[j].opt()],
                    outs=[table_q[j].opt()],
                )

            def quad_table_rows(q):
                return table_q[q][:, :]

            def myshard_write(b, tb):
                # block b local rows [b*128,(b+1)*128) -> quarter shard(s)
                lo = b * 128
                hi = lo + 128
                j = lo // QTR
                while lo < hi:
                    end = min(hi, (j + 1) * QTR)
                    nc.sync.dma_start(
                        myshard_q[j][lo - j * QTR:end - j * QTR, :],
                        tb[lo - b * 128:end - b * 128, :])
                    lo = end
                    j += 1

            own_store = {}

            def table_row_block(l, b, lhsT_ap):
                """GEMM + dinv scale + store to myshard rows of block b."""
                ps = gemmps.tile([128, 128], F32, tag="gemm")
                nc.tensor.matmul(ps[:], lhsT=lhsT_ap, rhs=w_sb[l][:],
                                 start=True, stop=True)
                tb = tblp.tile([128, 128], TBL_DT, tag="tbl",
                               name=f"tb_{l}_{b}")
                nc.scalar.mul(tb[:], ps[:], dinv_sb[:, b:b + 1])
                myshard_write(b, tb)
                own_store[(l, b)] = tb

            # ---- phase A: layer-1 table from x (chunked loads so the
            # GEMM stream and the first quarter-AllGather start early) ----
            XB = 25   # chunk = one AllGather quarter (blocks 0-24, ...)
            for c0 in range(0, B, XB):
                nbx = min(XB, B - c0)
                xq = xtp.tile([128, XB * 128], TBL_DT, tag="xt")
                nc.sync.dma_start(xq[:, 0:nbx * 128],
                                  xT_in[:, c0 * 128:(c0 + nbx) * 128])
                for jx in range(nbx):
                    b = c0 + jx
                    table_row_block(0, b, xq[:, jx * 128:(jx + 1) * 128])
                    if debug_stage != "phaseA":
                        for j in range(4):
                            if b == qtr_last_blk[j]:
                                do_allgather(j)

            # ---- layers ----
            if debug_stage in ("phaseA", "table1"):
                n_layers = 0
            elif debug_stage in ("layer1", "agg_only"):
                n_layers = 1
            else:
                n_layers = 3
            for l in range(n_layers):
                slabs = [None] * B
                psq = {}

                def note_tail_done(b):
                    # fire next layer's quarter-AllGather once every block
                    # of that quarter has written its myshard rows (tails
                    # run in ascending block order within quad 3)
                    if l < 2 and debug_stage != "agg_only":
                        for j in range(4):
                            if b == qtr_last_blk[j]:
                                do_allgather(j)

                def block_tail(b):
                    s = slabs[b]
                    if slab_dbg is not None:
                        nc.sync.dma_start(
                            slab_dbg[b * 128:(b + 1) * 128, :], s[:])
                    # h = relu(dinv * slab); bias already in psum via the
                    # rank-1 e0^T @ biasb matmul
                    if l == 2:
                        h = workp.tile([128, 128], F32, tag="h")
                        nc.scalar.activation(h[:], s[:],
                                             mybir.ActivationFunctionType.Relu,
                                             bias=0.0,
                                             scale=dinv_sb[:, b:b + 1])
                        nc.sync.dma_start(out_dram[b * 128:(b + 1) * 128, :],
                                          h[:])
                        return
                    h = workp.tile([128, 128], TBL_DT, tag="h")
                    nc.scalar.activation(h[:], s[:],
                                         mybir.ActivationFunctionType.Relu,
                                         bias=0.0,
                                         scale=dinv_sb[:, b:b + 1])
                    tp = tpps.tile([128, 128], TBL_DT, tag="tp")
                    nc.tensor.transpose(tp[:], h[:], identb_sb[:])
                    htt = htp.tile([128, 128], TBL_DT, tag="ht")
                    nc.scalar.copy(htt[:], tp[:])
                    table_row_block(l + 1, b, htt)
                    note_tail_done(b)

                for ci, (q, t0, ntl, p0, plist) in enumerate(calls):
                    g = gatp.tile([128, CALL_MAX_TILES, 128], TBL_DT, tag="g")
                    nc.gpsimd.dma_gather(
                        g[:, 0:ntl, :],
                        quad_table_rows(q),
                        idx_sb[:, t0 * 8:(t0 + ntl) * 8],
                        ntl * 128, ntl * 128, 128,
                        single_packet=(ntl * 128 <= 1024),
                        queue_num=ci % N_QUEUES,
                    )
                    npc = len(plist)
                    a_all = app.tile([128, parts_call_max, 128], TBL_DT,
                                     tag="a")
                    iota3 = iota_sb[:].rearrange("p (o f) -> p o f", o=1)
                    sid3 = sid_sb[:, p0:p0 + npc].rearrange(
                        "p (t o) -> p t o", o=1)
                    i_b, s_b = bass.broadcast_tensor_aps(iota3, sid3)
                    nc.vector.tensor_tensor(a_all[:, 0:npc, :], i_b, s_b,
                                            op=mybir.AluOpType.is_equal)
                    for (pl, tl, b, first, last) in plist:
                        a = a_all[:, pl, :]
                        if first:
                            psq[b] = aggps.tile([128, 128], F32, tag="agg",
                                                name=f"agg_{l}_{q}_{b}")
                        do_self = (first and q == 0
                                   and (l, b) in own_store)
                        nc.tensor.matmul(psq[b][:], lhsT=a,
                                         rhs=g[:, tl, :],
                                         start=first,
                                         stop=last and not do_self)
                        if do_self:
                            # self-loop term: psum += I^T @ own_rows
                            nc.tensor.matmul(psq[b][:], lhsT=identb_sb[:],
                                             rhs=own_store[(l, b)][:],
                                             start=False, stop=False)
                            # bias term: psum += e0^T @ biasb (row 0 = bias)
                            nc.tensor.matmul(psq[b][:], lhsT=e0b_sb[:],
                                             rhs=biasb_sb[l][:],
                                             start=False, stop=last)
                        if last:
                            if q == 0:
                                slabs[b] = slabp.tile([128, 128], F32,
                                                      tag="slab",
                                                      name=f"slab_{l}_{b}")
                                nc.scalar.copy(slabs[b][:], psq[b][:])
                            else:
                                nc.vector.tensor_tensor(
                                    slabs[b][:], slabs[b][:], psq[b][:],
                                    op=mybir.AluOpType.add)
                            if q == QUADS - 1:
                                block_tail(b)

    nc.compile()
    return nc


# ----------------------------------------------------------------------------
# Runner
# ----------------------------------------------------------------------------

def make_in_maps(prep, Ws, bs):
    iota = np.tile(np.arange(128, dtype=np.float32)[None, :], (128, 1))
    ident = np.eye(128, dtype=np.float32)
    e0b = np.zeros((128, 128), np.float32)
    e0b[0, :] = 1.0
    maps = []
    for k in range(NC):
        maps.append({
            "xT": prep["xT"][k].astype(TBL_NP),
            "W1": Ws[0].astype(TBL_NP),
            "W2": Ws[1].astype(TBL_NP),
            "W3": Ws[2].astype(TBL_NP),
            "Bt1": np.tile(bs[0][None, :], (128, 1)).astype(np.float32),
            "Bt2": np.tile(bs[1][None, :], (128, 1)).astype(np.float32),
            "Bt3": np.tile(bs[2][None, :], (128, 1)).astype(np.float32),
            "iota": iota.astype(TBL_NP),
            "ident": ident,
            "identb": ident.astype(TBL_NP),
            "e0b": e0b.astype(TBL_NP),
            "dinv": prep["dinv_wr"][k],
            "sid": prep["sid_wr"][k].astype(TBL_NP),
            "idx": prep["idx_wr"][k],
        })
    return maps


def assemble_output(prep, results):
    full = np.zeros((N, C), np.float32)
    for k in range(NC):
        nodes = prep["node_at"][k]
        real = nodes >= 0
        full[nodes[real]] = results[k]["out"][real]
    return full


_CACHE = {}


def run(inputs, trace=False, sim=False):
    from concourse.bass_utils import run_bass_kernel_spmd

    x = np.asarray(inputs["x"], np.float32)
    Ws = [np.asarray(inputs[f"W{i+1}"], np.float32) for i in range(3)]
    bs = [np.asarray(inputs[f"b{i+1}"], np.float32) for i in range(3)]

    prep = preprocess(x, inputs["edge_index"])
    ckey = ("nc", TBL_NP, prep["S"], prep["n_calls"])
    if ckey not in _CACHE:
        _CACHE[ckey] = build_nc(prep)
    nc = _CACHE[ckey]

    in_maps = make_in_maps(prep, Ws, bs)

    if sim:
        from concourse.bass_interp import MultiCoreSim
        msim = MultiCoreSim(nc, NC, trace=False, require_finite=False,
                            require_nnan=False)
        for k in range(NC):
            for name, arr in in_maps[k].items():
                msim.cores[k].tensor(name)[:] = arr
        msim.simulate(check_with_hw=False)
        results = [{"out": np.array(msim.cores[k].tensor("out"))}
                   for k in range(NC)]
        return assemble_output(prep, results), None

    if trace:
        _install_axon_profile_hook()
    res = run_bass_kernel_spmd(nc, in_maps, list(range(NC)), trace=trace)
    return assemble_output(prep, res.results), res


def kernel(**inputs):
    out, _ = run(inputs)
    return out



# revision 57
# speedup vs baseline: 1.0127x; 1.0127x over previous
"""3-layer GCN (DiffPool-style conv stack) on Trainium2, 8 NeuronCores.

Strategy (graph/data parallel, per sharding hint):
  - Nodes are permuted by degree and dealt round-robin to 8 cores
    (12544 local nodes each incl. dummy padding; 98 blocks of 128).
  - Edges partitioned by destination owner; per core the edge stream is
    grouped by src quadrant and densely packed in dst-block order (a
    gather tile may span blocks; each (tile, block) intersection is a
    "part" with its own one-hot column), so dma_gather indices fit int16.
  - Per layer: each core computes the table rows for its own nodes
    T = dinv * (H @ W) (node-major, bf16), FOUR quarter-AllGathers
    assemble the full table (each fires as soon as its quarter of the
    tails is written, hiding the collective under the gather stream),
    then per-edge rows are fetched with gpsimd dma_gather (1024
    idxs/call, single_packet, 4 SWDGE queues, 32KB/partition descriptor
    rings) and aggregated with one-hot selection matmuls into per-block
    PSUM, accumulated across quadrants in an SBUF slab, then
    scale+relu'd (fused, Scalar engine) into the next layer's input;
    bias enters PSUM via a rank-1 matmul.
"""

import sys
import types

sys.path.insert(0, "/opt/trn_rl_repo")

import numpy as np

N = 100000
C = 128
NC = 8
L = 12544           # local nodes per core (98 blocks of 128)
B = L // 128        # 98
NPAD = NC * L       # 100352
QUADS = 4
QROWS = NPAD // QUADS   # 25088 (< 32767, fits int16 gather index)
CALL_MAX_TILES = 8      # 1024 indices per call: single_packet needs <=64 descs/engine, and multi-packet drains slower
N_QUEUES = 4

import ml_dtypes

TBL_NP = ml_dtypes.bfloat16  # table dtype; np.float32 or ml_dtypes.bfloat16


def _install_axon_profile_hook():
    """run_bass_kernel_spmd(trace=True) needs antenv.axon_hooks, absent in
    this image; register the equivalent ctypes hook."""
    try:
        import antenv
        if getattr(antenv, "axon_hooks", None) is not None:
            return
        from trn_agent_boot.trn_boot import _ntff_profile_via_ctypes
        mod = types.ModuleType("antenv.axon_hooks")
        hook = _ntff_profile_via_ctypes("/opt/axon/libaxon_pjrt.so")
        mod.get_axon_ntff_profile_hook = lambda: hook
        mod.set_axon_ntff_profile_hook = lambda h: None
        sys.modules["antenv.axon_hooks"] = mod
        antenv.axon_hooks = mod
    except Exception:
        pass


# ----------------------------------------------------------------------------
# Host preprocessing
# ----------------------------------------------------------------------------

def preprocess(x, edge_index):
    """Build the static SPMD schedule + per-core input arrays."""
    x = np.asarray(x, np.float32)
    ei = np.asarray(edge_index, np.int64)
    # self-loops are NOT placed in the gather stream: each core owns its
    # nodes' table rows, so the self term dinv_i^2*(HW)_i is added on-chip
    # from a stashed copy (identity matmul). deg still counts them.
    src = ei[0]
    dst = ei[1]

    deg = (np.bincount(dst, minlength=N) + 1).astype(np.float32)
    dinv = (1.0 / np.sqrt(deg)).astype(np.float32)

    order = np.argsort(deg, kind="stable")
    rank = np.empty(N, np.int64)
    rank[order] = np.arange(N)
    core_of = rank % NC
    # stratified snake-deal of each core's degree-ordered nodes over its
    # blocks: every block samples the same degree profile, equalizing
    # per-(quad, block) edge counts across blocks AND cores (minimizes
    # gather-tile padding, which is pure wasted gather bandwidth).
    pos = rank // NC
    slot_of = (pos % B) * 128 + pos // B
    gnew = core_of * L + slot_of

    # original node for (core, slot); -1 for dummy slots
    node_at = -np.ones((NC, L), np.int64)
    node_at[core_of, slot_of] = np.arange(N)

    gsrc = gnew[src]
    gdst = gnew[dst]
    owner = gdst // L
    ldst = gdst % L
    # table row numbering: quarter-shard interleave so the table is
    # assembled by FOUR AllGathers (one per quad) that fire as soon as each
    # quarter of the tails is done and fully pipeline with the gather phase.
    #   quarter j = slot // QTR; row = j*QROWS + core*QTR + slot%QTR
    QTR = L // 4                       # 3136 rows per quarter-shard
    sc = gsrc // L
    ss = gsrc % L
    qj = ss // QTR
    trow = qj * QROWS + sc * QTR + (ss - qj * QTR)
    quad = trow // QROWS
    qidx = trow % QROWS
    blk = ldst // 128
    sid = ldst % 128

    # dense per-quad packing: per-(quad, block) segments padded only to the
    # max-over-cores count (no 128 rounding); tiles may span blocks. Each
    # (tile, block) intersection is a "part" carrying its own one-hot column.
    key = (owner * QUADS + quad) * B + blk
    cnt = np.bincount(key, minlength=NC * QUADS * B).reshape(NC, QUADS, B)
    seg_len = cnt.max(axis=0).astype(np.int64)          # [QUADS, B]

    seg_start = np.zeros((QUADS, B), np.int64)          # slot offset in quad
    quad_tiles = np.zeros(QUADS, np.int64)
    quad_tile0 = np.zeros(QUADS, np.int64)
    t = 0
    for q in range(QUADS):
        s = 0
        for b in range(B):
            seg_start[q, b] = s
            s += seg_len[q, b]
        quad_tile0[q] = t
        quad_tiles[q] = (s + 127) // 128
        t += quad_tiles[q]
    n_tiles = int(t)
    S = n_tiles * 128

    # parts ordered by (q, tile, b)
    part_first = np.zeros((QUADS, B), np.int64)
    t_lo_g = np.zeros((QUADS, B), np.int64)
    parts_by_tile = [[] for _ in range(n_tiles)]  # (part, b, first, last)
    np_idx = 0
    for q in range(QUADS):
        for b in range(B):
            st = int(seg_start[q, b])
            en = st + int(seg_len[q, b])
            en_t = int(quad_tiles[q]) * 128 if b == B - 1 else en
            lo = st // 128
            hi = (en_t - 1) // 128
            part_first[q, b] = np_idx
            t_lo_g[q, b] = quad_tile0[q] + lo
            for tt in range(lo, hi + 1):
                gt = int(quad_tile0[q]) + tt
                parts_by_tile[gt].append((np_idx, b, tt == lo, tt == hi))
                np_idx += 1
    n_parts = np_idx

    # calls: chunk each quad's tile range into <=CALL_MAX_TILES-tile calls;
    # per call, the covered parts [(part_local, tile_local, b, first, last)]
    calls = []
    for q in range(QUADS):
        off = 0
        while off < int(quad_tiles[q]):
            ntl = min(CALL_MAX_TILES, int(quad_tiles[q]) - off)
            gt0 = int(quad_tile0[q]) + off
            plist = []
            p0 = parts_by_tile[gt0][0][0]
            for tt in range(ntl):
                for (pi, b, first, last) in parts_by_tile[gt0 + tt]:
                    plist.append((pi - p0, tt, b, first, last))
            calls.append((q, gt0, ntl, p0, plist))
            off += ntl
    n_calls = len(calls)
    parts_call_max = max(len(c[4]) for c in calls)

    # per-core slot arrays; pad slots gather a valid (spread) row but carry
    # sid=-999 so their one-hot column is all zeros.
    eorder = np.lexsort((qidx, blk, quad, owner))
    so, sq, sb_, sqi, ssid = (owner[eorder], quad[eorder], blk[eorder],
                              qidx[eorder], sid[eorder])
    skey = key[eorder]
    grp_change = np.flatnonzero(np.diff(skey, prepend=-1))
    grp_starts = np.zeros(len(skey), np.int64)
    grp_starts[grp_change] = np.arange(len(skey))[grp_change]
    np.maximum.accumulate(grp_starts, out=grp_starts)
    ranks = np.arange(len(skey)) - grp_starts

    slot = quad_tile0[sq] * 128 + seg_start[sq, sb_] + ranks
    gt_e = slot // 128
    part_e = part_first[sq, sb_] + (gt_e - t_lo_g[sq, sb_])

    pad_rows = (np.arange(S, dtype=np.int64) * 97) % QROWS
    idx16 = np.tile(pad_rows.astype(np.int16)[None, :], (NC, 1))
    idx16[so, slot] = sqi.astype(np.int16)

    sidc = np.full((NC, 128, n_parts), -999.0, np.float32)
    sidc[so, slot % 128, part_e] = ssid.astype(np.float32)

    callcnt = np.tile(np.array([ntl * 128 for (_, _, ntl, _, _) in calls],
                               np.int32)[None, :], (NC, 1))

    # wrapped per-core arrays
    idx_wr = np.zeros((NC, 128, S // 16), np.int16)
    for k in range(NC):
        w16 = idx16[k].reshape(S // 16, 16).T            # [16, S/16]
        idx_wr[k] = np.tile(w16, (8, 1))
    sid_wr = sidc                                        # [NC, 128, n_parts]

    # per-core node-major inputs
    xT = np.zeros((NC, 128, L), np.float32)
    dinv_wr = np.zeros((NC, 128, B), np.float32)
    for k in range(NC):
        nodes = node_at[k]
        real = nodes >= 0
        xk = np.zeros((L, C), np.float32)
        xk[real] = x[nodes[real]]
        xT[k] = xk.T
        dk = np.zeros(L, np.float32)
        dk[real] = dinv[nodes[real]]
        dinv_wr[k] = dk.reshape(B, 128).T

    return dict(
        node_at=node_at, dinv=dinv, S=S, n_tiles=n_tiles, n_parts=n_parts,
        calls=calls, n_calls=n_calls, parts_call_max=parts_call_max,
        idx16=idx16, sidc=sidc, callcnt=callcnt,
        idx_wr=idx_wr, sid_wr=sid_wr, xT=xT, dinv_wr=dinv_wr,
    )


def numpy_model(prep, x, Ws, bs, tbl_dt=None):
    """Exact numpy emulation of the device algorithm (for validation)."""
    if tbl_dt is None:
        tbl_dt = TBL_NP
    node_at = prep["node_at"]
    dinv_wr = prep["dinv_wr"]

    # dinv per (core, local) in node-major
    dloc = np.stack([dinv_wr[k].T.reshape(L) for k in range(NC)])   # [NC, L]
    H = np.stack([prep["xT"][k].T for k in range(NC)])              # [NC, L, C]

    out = None
    QTR = L // 4
    for l in range(3):
        # table build (quarter-shard interleave)
        table = np.zeros((NPAD, C), tbl_dt)
        own = []
        for k in range(NC):
            tk = ((H[k].astype(np.float32) @ Ws[l])
                  * dloc[k][:, None]).astype(tbl_dt)
            own.append(tk)
            for j in range(4):
                table[j * QROWS + k * QTR:j * QROWS + (k + 1) * QTR] = \
                    tk[j * QTR:(j + 1) * QTR]

        # aggregation (parts-based)
        Hn = np.zeros((NC, L, C), np.float32)
        for k in range(NC):
            S_acc = np.zeros((L, C), np.float32)
            for (q, gt0, ntl, p0, plist) in prep["calls"]:
                for (pl, tt, b, first, last) in plist:
                    gt = gt0 + tt
                    rows = (prep["idx16"][k][gt * 128:(gt + 1) * 128]
                            .astype(np.int64) + q * QROWS)
                    sidv = prep["sidc"][k][:, p0 + pl]
                    valid = sidv >= 0
                    tgt = b * 128 + sidv[valid].astype(np.int64)
                    np.add.at(S_acc, tgt,
                              table[rows[valid]].astype(np.float32))
            S_acc += own[k].astype(np.float32)          # self-loop term
            z = S_acc * dloc[k][:, None] + bs[l][None, :]
            Hn[k] = np.maximum(z, 0.0)
        H = Hn
        out = H
    # assemble
    full = np.zeros((N, C), np.float32)
    for k in range(NC):
        real = node_at[k] >= 0
        full[node_at[k][real]] = out[k][real]
    return full


# ----------------------------------------------------------------------------
# Bass program
# ----------------------------------------------------------------------------

def build_nc(prep, tbl_dt_np=None, debug_stage=None):
    import concourse.bass as bass
    import concourse.mybir as mybir
    import concourse.tile as tile
    from concourse import bacc

    if tbl_dt_np is None:
        tbl_dt_np = TBL_NP
    TBL_DT = mybir.dt.from_np(np.dtype(tbl_dt_np))
    F32 = mybir.dt.float32

    S = prep["S"]
    n_tiles = prep["n_tiles"]
    n_parts = prep["n_parts"]
    calls = prep["calls"]
    n_calls = prep["n_calls"]
    parts_call_max = prep["parts_call_max"]

    nc = bacc.Bacc("TRN2", target_bir_lowering=False, debug=False,
                   num_devices=NC, num_swdge_queues=N_QUEUES,
                   dynamic_dma_scratch_size=2**15)

    # inputs (xT and W in table dtype: the whole GEMM path runs bf16)
    xT_in = nc.dram_tensor("xT", [128, L], TBL_DT, kind="ExternalInput")
    w_in = [nc.dram_tensor(f"W{i+1}", [128, 128], TBL_DT,
                           kind="ExternalInput")
            for i in range(3)]
    bias_in = [nc.dram_tensor(f"Bt{i+1}", [128, 128], F32, kind="ExternalInput")
               for i in range(3)]
    iota_in = nc.dram_tensor("iota", [128, 128], TBL_DT, kind="ExternalInput")
    ident_in = nc.dram_tensor("ident", [128, 128], F32, kind="ExternalInput")
    identb_in = nc.dram_tensor("identb", [128, 128], TBL_DT,
                               kind="ExternalInput")
    e0b_in = nc.dram_tensor("e0b", [128, 128], TBL_DT, kind="ExternalInput")
    dinv_in = nc.dram_tensor("dinv", [128, B], F32, kind="ExternalInput")
    sid_in = nc.dram_tensor("sid", [128, n_parts], TBL_DT,
                            kind="ExternalInput")
    idx_in = nc.dram_tensor("idx", [128, S // 16], mybir.dt.int16,
                            kind="ExternalInput")
    out_dram = nc.dram_tensor("out", [L, 128], F32, kind="ExternalOutput")
    tbl_dbg_in = None
    slab_dbg = None
    if debug_stage == "agg_only":
        tbl_dbg_in = nc.dram_tensor("tbl_dbg", [NPAD, 128], TBL_DT,
                                    kind="ExternalInput")
        slab_dbg = nc.dram_tensor("slab_dbg", [L, 128], F32,
                                  kind="ExternalOutput")
        g_dbg = nc.dram_tensor("g_dbg", [128, CALL_MAX_TILES * 128], TBL_DT,
                               kind="ExternalOutput")
        a_dbg = nc.dram_tensor("a_dbg", [128, 128], F32,
                               kind="ExternalOutput")

    from contextlib import ExitStack

    with tile.TileContext(nc) as tc, ExitStack() as es:
        constp = es.enter_context(tc.tile_pool(name="const", bufs=1))
        idxp = es.enter_context(tc.tile_pool(name="idxp", bufs=1))
        xtp = es.enter_context(tc.tile_pool(name="xt", bufs=2))
        gatp = es.enter_context(tc.tile_pool(name="gat", bufs=14))
        app = es.enter_context(tc.tile_pool(name="ap", bufs=8))
        slabp = es.enter_context(tc.tile_pool(name="slab", bufs=B))
        workp = es.enter_context(tc.tile_pool(name="work", bufs=4))
        tblp = es.enter_context(tc.tile_pool(name="tblp", bufs=B + 8))
        htp = es.enter_context(tc.tile_pool(name="htp", bufs=3))
        aggps = es.enter_context(tc.tile_pool(name="aggps", bufs=5, space="PSUM"))
        tpps = es.enter_context(tc.tile_pool(name="tpps", bufs=1, space="PSUM"))
        gemmps = es.enter_context(tc.tile_pool(name="gemmps", bufs=2, space="PSUM"))
        dramp = es.enter_context(tc.tile_pool(name="dram", bufs=1, space="DRAM"))
        if True:

            # ---- resident constants ----
            w_sb = []
            bias_sb = []
            for i in range(3):
                w = constp.tile([128, 128], TBL_DT, tag=f"w{i}")
                nc.sync.dma_start(w[:], w_in[i][:, :])
                w_sb.append(w)
                bb = constp.tile([128, 128], F32, tag=f"bias{i}")
                nc.sync.dma_start(bb[:], bias_in[i][:, :])
                bias_sb.append(bb)
            iota_sb = constp.tile([128, 128], TBL_DT, tag="iota")
            nc.sync.dma_start(iota_sb[:], iota_in[:, :])
            ident_sb = constp.tile([128, 128], F32, tag="ident")
            nc.sync.dma_start(ident_sb[:], ident_in[:, :])
            identb_sb = constp.tile([128, 128], TBL_DT, tag="identb")
            nc.sync.dma_start(identb_sb[:], identb_in[:, :])
            # row-0-ones bf16 tile (rank-1 bias matmul: psum += e0^T @ biasb)
            e0b_sb = constp.tile([128, 128], TBL_DT, tag="e0b")
            nc.sync.dma_start(e0b_sb[:], e0b_in[:, :])
            biasb_sb = []
            for i in range(3):
                bbq = constp.tile([128, 128], TBL_DT, tag=f"biasb{i}")
                nc.vector.tensor_copy(bbq[:], bias_sb[i][:])
                biasb_sb.append(bbq)
            dinv_sb = constp.tile([128, B], F32, tag="dinv")
            nc.sync.dma_start(dinv_sb[:], dinv_in[:, :])
            sid_sb = constp.tile([128, n_parts], TBL_DT, tag="sid")
            idx_sb = idxp.tile([128, S // 16], mybir.dt.int16, tag="idx")

            QTR = L // 4               # 3136 rows per quarter-shard
            # last block whose slots fall (partly) in quarter j: AG-j fires
            # once that block's myshard rows are written.
            qtr_last_blk = [((j + 1) * QTR - 1) // 128 for j in range(4)]
            myshard_q = [dramp.tile([QTR, 128], TBL_DT, tag=f"myshard{j}",
                                    name=f"myshard{j}")
                         for j in range(4)]
            table_q = [dramp.tile([QROWS, 128], TBL_DT, tag=f"table{j}",
                                  name=f"table{j}")
                       for j in range(4)]

            def do_allgather(j):
                nc.gpsimd.collective_compute(
                    "AllGather",
                    mybir.AluOpType.bypass,
                    replica_groups=[list(range(NC))],
                    ins=[myshard_q[j].opt()],
                    outs=[table_q[j].opt()],
                )

            def quad_table_rows(q):
                return table_q[q][:, :]

            def myshard_write(b, tb):
                # block b local rows [b*128,(b+1)*128) -> quarter shard(s)
                lo = b * 128
                hi = lo + 128
                j = lo // QTR
                while lo < hi:
                    end = min(hi, (j + 1) * QTR)
                    nc.sync.dma_start(
                        myshard_q[j][lo - j * QTR:end - j * QTR, :],
                        tb[lo - b * 128:end - b * 128, :])
                    lo = end
                    j += 1

            own_store = {}

            def table_row_block(l, b, lhsT_ap):
                """GEMM + dinv scale + store to myshard rows of block b."""
                ps = gemmps.tile([128, 128], F32, tag="gemm")
                nc.tensor.matmul(ps[:], lhsT=lhsT_ap, rhs=w_sb[l][:],
                                 start=True, stop=True)
                tb = tblp.tile([128, 128], TBL_DT, tag="tbl",
                               name=f"tb_{l}_{b}")
                nc.scalar.mul(tb[:], ps[:], dinv_sb[:, b:b + 1])
                myshard_write(b, tb)
                own_store[(l, b)] = tb

            # ---- phase A: layer-1 table from x (chunked loads so the
            # GEMM stream and the first quarter-AllGather start early) ----
            XB = 25   # chunk = one AllGather quarter (blocks 0-24, ...)
            for c0 in range(0, B, XB):
                nbx = min(XB, B - c0)
                xq = xtp.tile([128, XB * 128], TBL_DT, tag="xt")
                nc.sync.dma_start(xq[:, 0:nbx * 128],
                                  xT_in[:, c0 * 128:(c0 + nbx) * 128])
                if c0 == XB:
                    # idx/sid are first needed by the layer-1 gathers, which
                    # wait on AG-0 anyway: load them behind the first chunk
                    nc.sync.dma_start(idx_sb[:], idx_in[:, :])
                    nc.sync.dma_start(sid_sb[:], sid_in[:, :])
                for jx in range(nbx):
                    b = c0 + jx
                    table_row_block(0, b, xq[:, jx * 128:(jx + 1) * 128])
                    if debug_stage != "phaseA":
                        for j in range(4):
                            if b == qtr_last_blk[j]:
                                do_allgather(j)

            # ---- layers ----
            if debug_stage in ("phaseA", "table1"):
                n_layers = 0
            elif debug_stage in ("layer1", "agg_only"):
                n_layers = 1
            else:
                n_layers = 3
            for l in range(n_layers):
                slabs = [None] * B
                psq = {}

                def note_tail_done(b):
                    # fire next layer's quarter-AllGather once every block
                    # of that quarter has written its myshard rows (tails
                    # run in ascending block order within quad 3)
                    if l < 2 and debug_stage != "agg_only":
                        for j in range(4):
                            if b == qtr_last_blk[j]:
                                do_allgather(j)

                def block_tail(b):
                    s = slabs[b]
                    if slab_dbg is not None:
                        nc.sync.dma_start(
                            slab_dbg[b * 128:(b + 1) * 128, :], s[:])
                    # h = relu(dinv * slab); bias already in psum via the
                    # rank-1 e0^T @ biasb matmul
                    if l == 2:
                        h = workp.tile([128, 128], F32, tag="h")
                        nc.scalar.activation(h[:], s[:],
                                             mybir.ActivationFunctionType.Relu,
                                             bias=0.0,
                                             scale=dinv_sb[:, b:b + 1])
                        nc.sync.dma_start(out_dram[b * 128:(b + 1) * 128, :],
                                          h[:])
                        return
                    h = workp.tile([128, 128], TBL_DT, tag="h")
                    nc.scalar.activation(h[:], s[:],
                                         mybir.ActivationFunctionType.Relu,
                                         bias=0.0,
                                         scale=dinv_sb[:, b:b + 1])
                    tp = tpps.tile([128, 128], TBL_DT, tag="tp")
                    nc.tensor.transpose(tp[:], h[:], identb_sb[:])
                    htt = htp.tile([128, 128], TBL_DT, tag="ht")
                    nc.scalar.copy(htt[:], tp[:])
                    table_row_block(l + 1, b, htt)
                    note_tail_done(b)

                for ci, (q, t0, ntl, p0, plist) in enumerate(calls):
                    g = gatp.tile([128, CALL_MAX_TILES, 128], TBL_DT, tag="g")
                    nc.gpsimd.dma_gather(
                        g[:, 0:ntl, :],
                        quad_table_rows(q),
                        idx_sb[:, t0 * 8:(t0 + ntl) * 8],
                        ntl * 128, ntl * 128, 128,
                        single_packet=(ntl * 128 <= 1024),
                        queue_num=ci % N_QUEUES,
                    )
                    npc = len(plist)
                    a_all = app.tile([128, parts_call_max, 128], TBL_DT,
                                     tag="a")
                    iota3 = iota_sb[:].rearrange("p (o f) -> p o f", o=1)
                    sid3 = sid_sb[:, p0:p0 + npc].rearrange(
                        "p (t o) -> p t o", o=1)
                    i_b, s_b = bass.broadcast_tensor_aps(iota3, sid3)
                    nc.vector.tensor_tensor(a_all[:, 0:npc, :], i_b, s_b,
                                            op=mybir.AluOpType.is_equal)
                    for (pl, tl, b, first, last) in plist:
                        a = a_all[:, pl, :]
                        if first:
                            psq[b] = aggps.tile([128, 128], F32, tag="agg",
                                                name=f"agg_{l}_{q}_{b}")
                        do_self = (first and q == 0
                                   and (l, b) in own_store)
                        nc.tensor.matmul(psq[b][:], lhsT=a,
                                         rhs=g[:, tl, :],
                                         start=first,
                                         stop=last and not do_self)
                        if do_self:
                            # self-loop term: psum += I^T @ own_rows
                            nc.tensor.matmul(psq[b][:], lhsT=identb_sb[:],
                                             rhs=own_store[(l, b)][:],
                                             start=False, stop=False)
                            # bias term: psum += e0^T @ biasb (row 0 = bias)
                            nc.tensor.matmul(psq[b][:], lhsT=e0b_sb[:],
                                             rhs=biasb_sb[l][:],
                                             start=False, stop=last)
                        if last:
                            if q == 0:
                                slabs[b] = slabp.tile([128, 128], F32,
                                                      tag="slab",
                                                      name=f"slab_{l}_{b}")
                                nc.scalar.copy(slabs[b][:], psq[b][:])
                            else:
                                nc.vector.tensor_tensor(
                                    slabs[b][:], slabs[b][:], psq[b][:],
                                    op=mybir.AluOpType.add)
                            if q == QUADS - 1:
                                block_tail(b)

    nc.compile()
    return nc


# ----------------------------------------------------------------------------
# Runner
# ----------------------------------------------------------------------------

def make_in_maps(prep, Ws, bs):
    iota = np.tile(np.arange(128, dtype=np.float32)[None, :], (128, 1))
    ident = np.eye(128, dtype=np.float32)
    e0b = np.zeros((128, 128), np.float32)
    e0b[0, :] = 1.0
    maps = []
    for k in range(NC):
        maps.append({
            "xT": prep["xT"][k].astype(TBL_NP),
            "W1": Ws[0].astype(TBL_NP),
            "W2": Ws[1].astype(TBL_NP),
            "W3": Ws[2].astype(TBL_NP),
            "Bt1": np.tile(bs[0][None, :], (128, 1)).astype(np.float32),
            "Bt2": np.tile(bs[1][None, :], (128, 1)).astype(np.float32),
            "Bt3": np.tile(bs[2][None, :], (128, 1)).astype(np.float32),
            "iota": iota.astype(TBL_NP),
            "ident": ident,
            "identb": ident.astype(TBL_NP),
            "e0b": e0b.astype(TBL_NP),
            "dinv": prep["dinv_wr"][k],
            "sid": prep["sid_wr"][k].astype(TBL_NP),
            "idx": prep["idx_wr"][k],
        })
    return maps


def assemble_output(prep, results):
    full = np.zeros((N, C), np.float32)
    for k in range(NC):
        nodes = prep["node_at"][k]
        real = nodes >= 0
        full[nodes[real]] = results[k]["out"][real]
    return full


_CACHE = {}


def run(inputs, trace=False, sim=False):
    from concourse.bass_utils import run_bass_kernel_spmd

    x = np.asarray(inputs["x"], np.float32)
    Ws = [np.asarray(inputs[f"W{i+1}"], np.float32) for i in range(3)]
    bs = [np.asarray(inputs[f"b{i+1}"], np.float32) for i in range(3)]

    prep = preprocess(x, inputs["edge_index"])
    ckey = ("nc", TBL_NP, prep["S"], prep["n_calls"])
    if ckey not in _CACHE:
        _CACHE[ckey] = build_nc(prep)
    nc = _CACHE[ckey]

    in_maps = make_in_maps(prep, Ws, bs)

    if sim:
        from concourse.bass_interp import MultiCoreSim
        msim = MultiCoreSim(nc, NC, trace=False, require_finite=False,
                            require_nnan=False)
        for k in range(NC):
            for name, arr in in_maps[k].items():
                msim.cores[k].tensor(name)[:] = arr
        msim.simulate(check_with_hw=False)
        results = [{"out": np.array(msim.cores[k].tensor("out"))}
                   for k in range(NC)]
        return assemble_output(prep, results), None

    if trace:
        _install_axon_profile_hook()
    res = run_bass_kernel_spmd(nc, in_maps, list(range(NC)), trace=trace)
    return assemble_output(prep, res.results), res


def kernel(**inputs):
    out, _ = run(inputs)
    return out

